# revision 30
# baseline (speedup 1.0000x reference)
"""Routed (sparse) MoE kernel for Trainium2, expert-parallel over 8 NeuronCores.

Problem: Qwen3-MoE sparse block. T=2048 tokens, H=2048 hidden, E=32 experts,
F=768 intermediate, top-K=8, norm_topk_prob=True.

Strategy:
  * Host: router (logits -> softmax -> top-8 -> renormalize), replicated with
    jax-on-CPU to match the reference's numerics bit-for-bit where possible.
  * Host: gather each expert's routed tokens into a fixed-capacity (512) slot,
    packed partition-major, cast to bf16. Expert e -> core e%8, slot e//8.
    Tokens beyond capacity (rare: mean count is 512) are computed on host in
    fp32 — this keeps the device graph shape input-independent.
  * Device (per core): 4 expert slots. For each slot, the whole SwiGLU FFN in
    a transposed dataflow (tokens on the matmul free axis), bf16 matmuls with
    fp32 PSUM accumulation, silu on ACT, multiply on DVE:
        gT[F,C] = Wg^T x      (lhsT = Wg[H,F] tiles, rhs = xT[H,C] tiles)
        uT[F,C] = Wu^T x
        hT      = silu(gT) * uT
        yT[H,C] = Wd^T h      (lhsT = Wd[F,H] tiles, rhs = hT tiles)
    No on-chip transposes anywhere.
  * All DRAM tensors are packed partition-major on the host ([.., P, free]) so
    every DMA is a dense 2D pattern (cheap queue issue, few descriptors). All
    loads go on the ONE sync HWDGE ring in exact need-order — FIFO on the ring
    means the bytes the PE needs next are always the bytes in flight. Every
    slot's gate phase is k-outer (6 persistent PSUM banks, one per F-tile),
    consuming each k-chunk as it lands: slot 0 starts on a 1-k-tile head chunk
    ~8us in instead of waiting for the full 5MB preload (PE warm-up matmuls
    bridge the lead-in and trip the HAM clock-gate), and slot s+1's gate is
    emitted INSIDE slot s's down phase so the PE bridges slot boundaries while
    trailing y stores and late chunks complete. Weight prefetch runs one slot
    ahead from the up phase.
  * Host: combine — out[t] = sum_k w[t,k] * y_{e_k}[t], a per-expert weighted
    scatter-add with unique indices (fp32).
"""

import numpy as np
import ml_dtypes

import concourse.bass as bass  # noqa: F401  (registers engines)
import concourse.mybir as mybir
import concourse.tile as tile
from concourse import bacc
from concourse.bass_utils import run_bass_kernel_spmd

# Model dims (hardcoded per problem spec)
T, H, E, F, K = 2048, 2048, 32, 768, 8
NCORES = 8
SLOTS = E // NCORES  # 4 expert slots per core
C = 512              # per-expert token capacity on device
P = 128
KH = H // P          # 16 k-tiles over hidden
MF = F // P          # 6  m-tiles over intermediate
KF = F // P          # 6  k-tiles over intermediate (down proj)
MH = H // P          # 16 m-tiles over hidden (down proj)
G = 4                # y-store batch (mh tiles per DMA)

BF16 = mybir.dt.bfloat16
F32 = mybir.dt.float32

# Exposed for test harnesses: the BassKernelResults of the last device run.
LAST_RESULT = None

_NC_CACHE = None

# k-chunking of the hidden dim for x / w_gate / w_up loads. Slot 0's head
# chunks are tiny so the k-outer gate phase can start as soon as the first
# k-tile lands; steady-state chunks are big so the per-DMA queue issue cost
# amortizes. (nk -> bufs) is sized so every slot-0 chunk stays resident
# through the up phase.
CHUNKS0 = [(0, 1), (1, 1), (2, 2), (4, 4), (8, 4), (12, 4)]
CHUNKS = [(0, 8), (8, 8)]
WUCHUNKS = [(0, 4), (4, 4), (8, 4), (12, 4)]
NBUFS = {1: 2, 2: 1, 4: 3, 8: 4}


def _k2chunk(chunks):
    m = {}
    for ci, (k0, nk) in enumerate(chunks):
        for k in range(k0, k0 + nk):
            m[k] = (ci, k - k0)
    return m


K2CHUNK0 = _k2chunk(CHUNKS0)
K2CHUNKS = _k2chunk(CHUNKS)
K2WU = _k2chunk(WUCHUNKS)


def _build_graph():
    """One SPMD graph, identical for all 8 cores (only input data differs)."""
    nc = bacc.Bacc("TRN2", target_bir_lowering=False, debug=False,
                   num_devices=NCORES)
    # Partition-major packing: [SLOTS, P, free] — every DMA below is a dense
    # 2D access (128 partitions x one contiguous run), no rearrange needed.
    xt_d = nc.dram_tensor("xt", [SLOTS, P, KH * C], BF16, kind="ExternalInput").ap()
    wg_d = nc.dram_tensor("wg", [SLOTS, P, KH * F], BF16, kind="ExternalInput").ap()
    wu_d = nc.dram_tensor("wu", [SLOTS, P, KH * F], BF16, kind="ExternalInput").ap()
    wd_d = nc.dram_tensor("wd", [SLOTS, P, KF * H], BF16, kind="ExternalInput").ap()
    y_d = nc.dram_tensor("y", [SLOTS, P, MH * C], BF16, kind="ExternalOutput").ap()

    DCH = 3   # k-tiles per wd load chunk

    with tile.TileContext(nc) as tc:
        with (
            tc.tile_pool(name="warm", bufs=1) as warm,
            tc.tile_pool(name="xp", bufs=3) as xp,
            tc.tile_pool(name="wgp", bufs=3) as wgp,
            tc.tile_pool(name="wup", bufs=3) as wup,
            tc.tile_pool(name="wdp", bufs=2) as wdp,
            tc.tile_pool(name="hp", bufs=MF) as hp,
            tc.tile_pool(name="sp", bufs=2) as sp,
            tc.tile_pool(name="yp", bufs=4) as yp,
            tc.tile_pool(name="psA", bufs=MF, space="PSUM") as psA,
            tc.tile_pool(name="ps", bufs=2, space="PSUM") as ps,
        ):
            # All loads ride the sync HWDGE ring in need-order; y stores go
            # out on the (otherwise idle) gpsimd SWDGE ring so store issue
            # and completion never queue behind loads.
            def load_x_wg(s):
                chunks = CHUNKS0 if s == 0 else CHUNKS
                x_t, wg_t = [], []
                for ci, (k0, nk) in enumerate(chunks):
                    nb = NBUFS[nk]
                    # Slot 0's gate is DMA-paced and each ring tops out around
                    # ~240 GB/s: alternate the (x, wg) pair of each chunk
                    # across the sync and scalar HWDGE rings so the two rings
                    # carry equal bytes and every chunk lands before the
                    # k-outer gate reaches it. Steady-state slots stay
                    # all-sync (prefetched a slot ahead, never tight).
                    if s == 0:
                        x_eng = nc.sync if ci % 2 == 0 else nc.scalar
                        wg_eng = nc.scalar if ci % 2 == 0 else nc.sync
                    else:
                        x_eng = wg_eng = nc.sync
                    xc = xp.tile([P, nk * C], BF16, tag=f"x{nk}", bufs=nb)
                    x_eng.dma_start(xc[:], xt_d[s, :, k0 * C:(k0 + nk) * C])
                    x_t.append(xc)
                    wc = wgp.tile([P, nk * F], BF16, tag=f"wg{nk}", bufs=nb)
                    wg_eng.dma_start(wc[:], wg_d[s, :, k0 * F:(k0 + nk) * F])
                    wg_t.append(wc)
                return x_t, wg_t

            def load_wu(s):
                wu_t = []
                for ci, (k0, nk) in enumerate(WUCHUNKS):
                    wc = wup.tile([P, nk * F], BF16, tag="wu4", bufs=4)
                    # Slot 0's first wu chunks ride the scalar ring (idle
                    # once the gate chunks are in) so the up phase never
                    # waits; steady-state wu trails loads on sync.
                    eng = nc.scalar if (s == 0 and ci < 2) else nc.sync
                    eng.dma_start(wc[:], wu_d[s, :, k0 * F:(k0 + nk) * F])
                    wu_t.append(wc)
                return wu_t

            def load_wd(s):
                wd_t = []
                for c in range(KF // DCH):
                    wc = wdp.tile([P, DCH * H], BF16, tag="wd3", bufs=2)
                    nc.sync.dma_start(
                        wc[:], wd_d[s, :, c * DCH * H:(c + 1) * DCH * H]
                    )
                    wd_t.append(wc)
                return wd_t

            # PE warm-up tiles: memset on the vector queue (its ring-init
            # finishes earliest and it has no other work this early)
            wlhs = warm.tile([P, P], BF16, tag="wlhs")
            wrhs = warm.tile([P, C], BF16, tag="wrhs")
            nc.vector.memset(wlhs[:], 0.0)
            nc.vector.memset(wrhs[:], 0.0)

            # Slot 0 loads first, in exact need-order; wu/wd trail on the
            # same FIFO ring so they never steal bandwidth from the gate path.
            x_t0, wg_t0 = load_x_wg(0)
            wu_t0 = load_wu(0)

            # Warm-up matmuls bridge the DMA lead-in and trip the HAM window.
            wps = psA.tile([P, C], F32, tag="psg")
            for _ in range(7):
                nc.tensor.matmul(wps[:], wlhs[:], wrhs[:], start=True, stop=True)

            def emit_gate_kouter(s, x_t, wg_t, k2c):
                """k-outer gate: 6 persistent PSUM banks, consume each k-chunk
                as it lands. Needs only the first chunk to start."""
                psg6 = [psA.tile([P, C], F32, tag="psg", name=f"psg_{s}_{m}")
                        for m in range(MF)]
                for k in range(KH):
                    ci, off = k2c[k]
                    for m in range(MF):
                        nc.tensor.matmul(
                            psg6[m][:],
                            wg_t[ci][:, off * F + m * P: off * F + (m + 1) * P],
                            x_t[ci][:, off * C:(off + 1) * C],
                            start=(k == 0), stop=(k == KH - 1),
                        )
                return psg6

            # ---- Slot 0 head: gate straight off the head chunks ----
            wd_t = load_wd(0)
            cur = (x_t0, wu_t0, wd_t, K2CHUNK0,
                   emit_gate_kouter(0, x_t0, wg_t0, K2CHUNK0))
            nxt = None

            for s in range(SLOTS):
                x_t, wu_t, wd_t, k2c, psg6 = cur

                # up projection + silu + mult
                h_tiles = []
                for m in range(MF):
                    psu = ps.tile([P, C], F32, tag="pp")
                    for k in range(KH):
                        ci, off = k2c[k]
                        nc.tensor.matmul(
                            psu[:],
                            wu_t[K2WU[k][0]][
                                :, K2WU[k][1] * F + m * P:
                                K2WU[k][1] * F + (m + 1) * P],
                            x_t[ci][:, off * C:(off + 1) * C],
                            start=(k == 0), stop=(k == KH - 1),
                        )
                    sil = sp.tile([P, C], F32, tag="sil")
                    nc.scalar.activation(
                        sil[:], psg6[m][:], mybir.ActivationFunctionType.Silu
                    )
                    hm = hp.tile([P, C], BF16, tag="h")
                    nc.vector.tensor_tensor(
                        hm[:], sil[:], psu[:], mybir.AluOpType.mult
                    )
                    h_tiles.append(hm)
                    if m == 1 and s + 1 < SLOTS:
                        nxt = load_x_wg(s + 1) + (load_wu(s + 1),)

                # down projection; y stores batched G mh-tiles per DMA from
                # the scalar queue. The NEXT slot's k-outer gate is emitted
                # inside the down phase (after mh==11) so the PE bridges the
                # slot boundary while this slot's trailing y stores and the
                # next slot's late chunks complete.
                yt = None
                for mh in range(MH):
                    if mh == 12 and s + 1 < SLOTS:
                        nx_t, nwg_t, nwu_t = nxt
                        nwd_t = load_wd(s + 1)
                        cur = (nx_t, nwu_t, nwd_t, K2CHUNKS,
                               emit_gate_kouter(s + 1, nx_t, nwg_t, K2CHUNKS))
                    psy = ps.tile([P, C], F32, tag="pp")
                    for k in range(KF):
                        nc.tensor.matmul(
                            psy[:],
                            wd_t[k // DCH][:, (k % DCH) * H + mh * P:(k % DCH) * H + (mh + 1) * P],
                            h_tiles[k][:],
                            start=(k == 0), stop=(k == KF - 1),
                        )
                    g_here = G if s + 1 < SLOTS else G // 2
                    j = mh % g_here
                    if j == 0:
                        yt = yp.tile([P, G * C], BF16, tag="y", bufs=3)
                    nc.vector.tensor_copy(out=yt[:, j * C:(j + 1) * C], in_=psy[:])
                    if j == g_here - 1:
                        g0 = mh - (g_here - 1)
                        # Mid-kernel stores ride the idle gpsimd SWDGE ring
                        # (never queue behind loads). The last slot's stores
                        # use the scalar HWDGE ring instead: loads are done
                        # by then, and keeping SWDGE idle at the end makes
                        # the epilogue's gpsimd drain instant.
                        eng = nc.gpsimd if s + 1 < SLOTS else nc.scalar
                        eng.dma_start(
                            y_d[s, :, g0 * C:(g0 + g_here) * C],
                            yt[:, : g_here * C],
                        )

    nc.compile()
    return nc


def _route(x, gate_w):
    """Replicate the reference router. Returns (topk_idx, topk_w) as numpy."""
    try:
        import jax
        import jax.numpy as jnp

        cpu = jax.devices("cpu")[0]
        with jax.default_device(cpu):
            logits = jnp.asarray(x) @ jnp.asarray(gate_w)
            probs = jax.nn.softmax(logits.astype(jnp.float32), axis=-1)
            topk_w, topk_idx = jax.lax.top_k(probs, K)
            topk_w = topk_w / jnp.sum(topk_w, axis=-1, keepdims=True)
            return np.asarray(topk_idx), np.asarray(topk_w)
    except Exception:
        logits = x.astype(np.float32) @ gate_w.astype(np.float32)
        lm = logits.max(-1, keepdims=True)
        p = np.exp(logits - lm)
        p /= p.sum(-1, keepdims=True)
        topk_idx = np.argsort(-p, kind="stable", axis=-1)[:, :K]
        topk_w = np.take_along_axis(p, topk_idx, axis=-1)
        topk_w = topk_w / topk_w.sum(-1, keepdims=True)
        return topk_idx.astype(np.int32), topk_w


def _silu(v):
    return v / (1.0 + np.exp(-v))


def _pack_pm(a, kt):
    """[S, kt*P, free] -> partition-major [S, P, kt*free]."""
    s, rows, free = a.shape
    return np.ascontiguousarray(
        a.reshape(s, kt, P, free).transpose(0, 2, 1, 3).reshape(s, P, kt * free)
    )


def kernel(hidden_states, gate_w, w_gate_proj, w_up_proj, w_down_proj):
    global LAST_RESULT, _NC_CACHE

    x = np.asarray(hidden_states, dtype=np.float32)
    gate_w = np.asarray(gate_w, dtype=np.float32)
    wg_all = np.asarray(w_gate_proj, dtype=np.float32)
    wu_all = np.asarray(w_up_proj, dtype=np.float32)
    wd_all = np.asarray(w_down_proj, dtype=np.float32)

    # ---- Host router ----
    topk_idx, topk_w = _route(x, gate_w)

    # Per-expert token lists (kept on device up to capacity C; rest on host)
    route_w = np.zeros((T, E), np.float32)
    np.put_along_axis(route_w, topk_idx, topk_w.astype(np.float32), axis=-1)
    expert_tokens = [np.nonzero(route_w[:, e])[0] for e in range(E)]

    x_bf = x.astype(ml_dtypes.bfloat16)

    # ---- Build per-core inputs (partition-major packing) ----
    in_maps = []
    for core in range(NCORES):
        experts = [core + NCORES * s for s in range(SLOTS)]
        xt = np.zeros((SLOTS, H, C), ml_dtypes.bfloat16)
        for s, e in enumerate(experts):
            idx = expert_tokens[e][:C]
            xt[s, :, : len(idx)] = x_bf[idx].T
        in_maps.append(
            {
                "xt": _pack_pm(xt, KH),
                "wg": _pack_pm(wg_all[experts].astype(ml_dtypes.bfloat16), KH),
                "wu": _pack_pm(wu_all[experts].astype(ml_dtypes.bfloat16), KH),
                "wd": _pack_pm(wd_all[experts].astype(ml_dtypes.bfloat16), KF),
            }
        )

    # ---- Device run ----
    if _NC_CACHE is None:
        _NC_CACHE = _build_graph()
    nc = _NC_CACHE
    res = run_bass_kernel_spmd(nc, in_maps, core_ids=list(range(NCORES)))
    LAST_RESULT = res

    # ---- Host combine ----
    out = np.zeros((T, H), np.float32)
    for e in range(E):
        core, s = e % NCORES, e // NCORES
        idx = expert_tokens[e]
        kept, ov = idx[:C], idx[C:]
        ysl = np.asarray(res.results[core]["y"][s])  # [P, MH*C] bf16
        yT = (
            ysl.reshape(P, MH, C).transpose(1, 0, 2).reshape(H, C)
            .astype(np.float32)
        )
        w_kept = route_w[kept, e]
        out[kept] += w_kept[:, None] * yT[:, : len(kept)].T
        if len(ov):
            xo = x[ov]
            h = _silu(xo @ wg_all[e]) * (xo @ wu_all[e])
            out[ov] += route_w[ov, e][:, None] * (h @ wd_all[e])

    return out


# revision 31
# speedup vs baseline: 1.0094x; 1.0094x over previous
"""Routed (sparse) MoE kernel for Trainium2, expert-parallel over 8 NeuronCores.

Problem: Qwen3-MoE sparse block. T=2048 tokens, H=2048 hidden, E=32 experts,
F=768 intermediate, top-K=8, norm_topk_prob=True.

Strategy:
  * Host: router (logits -> softmax -> top-8 -> renormalize), replicated with
    jax-on-CPU to match the reference's numerics bit-for-bit where possible.
  * Host: gather each expert's routed tokens into a fixed-capacity (512) slot,
    packed partition-major, cast to bf16. Expert e -> core e%8, slot e//8.
    Tokens beyond capacity (rare: mean count is 512) are computed on host in
    fp32 — this keeps the device graph shape input-independent.
  * Device (per core): 4 expert slots. For each slot, the whole SwiGLU FFN in
    a transposed dataflow (tokens on the matmul free axis), bf16 matmuls with
    fp32 PSUM accumulation, silu on ACT, multiply on DVE:
        gT[F,C] = Wg^T x      (lhsT = Wg[H,F] tiles, rhs = xT[H,C] tiles)
        uT[F,C] = Wu^T x
        hT      = silu(gT) * uT
        yT[H,C] = Wd^T h      (lhsT = Wd[F,H] tiles, rhs = hT tiles)
    No on-chip transposes anywhere.
  * All DRAM tensors are packed partition-major on the host ([.., P, free]) so
    every DMA is a dense 2D pattern (cheap queue issue, few descriptors). All
    loads go on the ONE sync HWDGE ring in exact need-order — FIFO on the ring
    means the bytes the PE needs next are always the bytes in flight. Every
    slot's gate phase is k-outer (6 persistent PSUM banks, one per F-tile),
    consuming each k-chunk as it lands: slot 0 starts on a 1-k-tile head chunk
    ~8us in instead of waiting for the full 5MB preload (PE warm-up matmuls
    bridge the lead-in and trip the HAM clock-gate), and slot s+1's gate is
    emitted INSIDE slot s's down phase so the PE bridges slot boundaries while
    trailing y stores and late chunks complete. Weight prefetch runs one slot
    ahead from the up phase.
  * Host: combine — out[t] = sum_k w[t,k] * y_{e_k}[t], a per-expert weighted
    scatter-add with unique indices (fp32).
"""

import numpy as np
import ml_dtypes

import concourse.bass as bass  # noqa: F401  (registers engines)
import concourse.mybir as mybir
import concourse.tile as tile
from concourse import bacc
from concourse.bass_utils import run_bass_kernel_spmd

# Model dims (hardcoded per problem spec)
T, H, E, F, K = 2048, 2048, 32, 768, 8
NCORES = 8
SLOTS = E // NCORES  # 4 expert slots per core
C = 512              # per-expert token capacity on device
P = 128
KH = H // P          # 16 k-tiles over hidden
MF = F // P          # 6  m-tiles over intermediate
KF = F // P          # 6  k-tiles over intermediate (down proj)
MH = H // P          # 16 m-tiles over hidden (down proj)
G = 4                # y-store batch (mh tiles per DMA)

BF16 = mybir.dt.bfloat16
F32 = mybir.dt.float32

# Exposed for test harnesses: the BassKernelResults of the last device run.
LAST_RESULT = None

_NC_CACHE = None

# k-chunking of the hidden dim for x / w_gate / w_up loads. Slot 0's head
# chunks are tiny so the k-outer gate phase can start as soon as the first
# k-tile lands; steady-state chunks are big so the per-DMA queue issue cost
# amortizes. (nk -> bufs) is sized so every slot-0 chunk stays resident
# through the up phase.
CHUNKS0 = [(0, 1), (1, 1), (2, 2), (4, 4), (8, 4), (12, 4)]
CHUNKS = [(0, 8), (8, 8)]
WUCHUNKS = [(0, 4), (4, 4), (8, 4), (12, 4)]
NBUFS = {1: 2, 2: 1, 4: 3, 8: 4}


def _k2chunk(chunks):
    m = {}
    for ci, (k0, nk) in enumerate(chunks):
        for k in range(k0, k0 + nk):
            m[k] = (ci, k - k0)
    return m


K2CHUNK0 = _k2chunk(CHUNKS0)
K2CHUNKS = _k2chunk(CHUNKS)
K2WU = _k2chunk(WUCHUNKS)


def _build_graph():
    """One SPMD graph, identical for all 8 cores (only input data differs)."""
    nc = bacc.Bacc("TRN2", target_bir_lowering=False, debug=False,
                   num_devices=NCORES)
    # Partition-major packing: [SLOTS, P, free] — every DMA below is a dense
    # 2D access (128 partitions x one contiguous run), no rearrange needed.
    xt_d = nc.dram_tensor("xt", [SLOTS, P, KH * C], BF16, kind="ExternalInput").ap()
    wg_d = nc.dram_tensor("wg", [SLOTS, P, KH * F], BF16, kind="ExternalInput").ap()
    wu_d = nc.dram_tensor("wu", [SLOTS, P, KH * F], BF16, kind="ExternalInput").ap()
    wd_d = nc.dram_tensor("wd", [SLOTS, P, KF * H], BF16, kind="ExternalInput").ap()
    y_d = nc.dram_tensor("y", [SLOTS, P, MH * C], BF16, kind="ExternalOutput").ap()

    DCH = 3   # k-tiles per wd load chunk

    with tile.TileContext(nc) as tc:
        with (
            tc.tile_pool(name="warm", bufs=1) as warm,
            tc.tile_pool(name="xp", bufs=3) as xp,
            tc.tile_pool(name="wgp", bufs=3) as wgp,
            tc.tile_pool(name="wup", bufs=3) as wup,
            tc.tile_pool(name="wdp", bufs=2) as wdp,
            tc.tile_pool(name="hp", bufs=MF) as hp,
            tc.tile_pool(name="sp", bufs=2) as sp,
            tc.tile_pool(name="yp", bufs=4) as yp,
            tc.tile_pool(name="psA", bufs=MF, space="PSUM") as psA,
            tc.tile_pool(name="ps", bufs=2, space="PSUM") as ps,
        ):
            # All loads ride the sync HWDGE ring in need-order; y stores go
            # out on the (otherwise idle) gpsimd SWDGE ring so store issue
            # and completion never queue behind loads.
            def load_x_wg(s):
                chunks = CHUNKS0 if s == 0 else CHUNKS
                x_t, wg_t = [], []
                for ci, (k0, nk) in enumerate(chunks):
                    nb = NBUFS[nk]
                    # Slot 0's gate is DMA-paced and each ring tops out around
                    # ~240 GB/s: alternate the (x, wg) pair of each chunk
                    # across the sync and scalar HWDGE rings so the two rings
                    # carry equal bytes and every chunk lands before the
                    # k-outer gate reaches it. Steady-state slots stay
                    # all-sync (prefetched a slot ahead, never tight).
                    if s == 0:
                        x_eng = nc.sync if ci % 2 == 0 else nc.scalar
                        wg_eng = nc.scalar if ci % 2 == 0 else nc.sync
                    else:
                        x_eng = wg_eng = nc.sync
                    xc = xp.tile([P, nk * C], BF16, tag=f"x{nk}", bufs=nb)
                    x_eng.dma_start(xc[:], xt_d[s, :, k0 * C:(k0 + nk) * C])
                    x_t.append(xc)
                    wc = wgp.tile([P, nk * F], BF16, tag=f"wg{nk}", bufs=nb)
                    wg_eng.dma_start(wc[:], wg_d[s, :, k0 * F:(k0 + nk) * F])
                    wg_t.append(wc)
                return x_t, wg_t

            def load_wu(s):
                wu_t = []
                for k0, nk in WUCHUNKS:
                    wc = wup.tile([P, nk * F], BF16, tag="wu4", bufs=6)
                    nc.sync.dma_start(wc[:], wu_d[s, :, k0 * F:(k0 + nk) * F])
                    wu_t.append(wc)
                return wu_t

            def load_wd(s):
                wd_t = []
                for c in range(KF // DCH):
                    wc = wdp.tile([P, DCH * H], BF16, tag="wd3", bufs=2)
                    nc.sync.dma_start(
                        wc[:], wd_d[s, :, c * DCH * H:(c + 1) * DCH * H]
                    )
                    wd_t.append(wc)
                return wd_t

            # PE warm-up tiles: memset on the vector queue (its ring-init
            # finishes earliest and it has no other work this early)
            wlhs = warm.tile([P, P], BF16, tag="wlhs")
            wrhs = warm.tile([P, C], BF16, tag="wrhs")
            nc.vector.memset(wlhs[:], 0.0)
            nc.vector.memset(wrhs[:], 0.0)

            # Slot 0 loads first, in exact need-order; wu/wd trail on the
            # same FIFO ring so they never steal bandwidth from the gate path.
            x_t0, wg_t0 = load_x_wg(0)
            wu_t0 = load_wu(0)

            # Warm-up matmuls bridge the DMA lead-in and trip the HAM window.
            wps = psA.tile([P, C], F32, tag="psg")
            for _ in range(7):
                nc.tensor.matmul(wps[:], wlhs[:], wrhs[:], start=True, stop=True)

            def emit_gate_kouter(s, x_t, wg_t, k2c):
                """k-outer gate: 6 persistent PSUM banks, consume each k-chunk
                as it lands. Needs only the first chunk to start."""
                psg6 = [psA.tile([P, C], F32, tag="psg", name=f"psg_{s}_{m}")
                        for m in range(MF)]
                for k in range(KH):
                    ci, off = k2c[k]
                    for m in range(MF):
                        nc.tensor.matmul(
                            psg6[m][:],
                            wg_t[ci][:, off * F + m * P: off * F + (m + 1) * P],
                            x_t[ci][:, off * C:(off + 1) * C],
                            start=(k == 0), stop=(k == KH - 1),
                        )
                return psg6

            # ---- Slot 0 head: gate straight off the head chunks ----
            wd_t = load_wd(0)
            cur = (x_t0, wu_t0, wd_t, K2CHUNK0,
                   emit_gate_kouter(0, x_t0, wg_t0, K2CHUNK0))
            nxt = None

            for s in range(SLOTS):
                x_t, wu_t, wd_t, k2c, psg6 = cur

                # up projection + silu + mult
                h_tiles = []
                for m in range(MF):
                    psu = ps.tile([P, C], F32, tag="pp")
                    for k in range(KH):
                        ci, off = k2c[k]
                        nc.tensor.matmul(
                            psu[:],
                            wu_t[K2WU[k][0]][
                                :, K2WU[k][1] * F + m * P:
                                K2WU[k][1] * F + (m + 1) * P],
                            x_t[ci][:, off * C:(off + 1) * C],
                            start=(k == 0), stop=(k == KH - 1),
                        )
                    sil = sp.tile([P, C], F32, tag="sil")
                    nc.scalar.activation(
                        sil[:], psg6[m][:], mybir.ActivationFunctionType.Silu
                    )
                    hm = hp.tile([P, C], BF16, tag="h")
                    nc.vector.tensor_tensor(
                        hm[:], sil[:], psu[:], mybir.AluOpType.mult
                    )
                    h_tiles.append(hm)
                    if m == 1 and s + 1 < SLOTS:
                        nxt = load_x_wg(s + 1) + (load_wu(s + 1),)

                # down projection; y stores batched G mh-tiles per DMA from
                # the scalar queue. The NEXT slot's k-outer gate is emitted
                # inside the down phase (after mh==11) so the PE bridges the
                # slot boundary while this slot's trailing y stores and the
                # next slot's late chunks complete.
                yt = None
                for mh in range(MH):
                    if mh == 12 and s + 1 < SLOTS:
                        nx_t, nwg_t, nwu_t = nxt
                        nwd_t = load_wd(s + 1)
                        cur = (nx_t, nwu_t, nwd_t, K2CHUNKS,
                               emit_gate_kouter(s + 1, nx_t, nwg_t, K2CHUNKS))
                    psy = ps.tile([P, C], F32, tag="pp")
                    for k in range(KF):
                        nc.tensor.matmul(
                            psy[:],
                            wd_t[k // DCH][:, (k % DCH) * H + mh * P:(k % DCH) * H + (mh + 1) * P],
                            h_tiles[k][:],
                            start=(k == 0), stop=(k == KF - 1),
                        )
                    g_here = G if s + 1 < SLOTS else G // 2
                    j = mh % g_here
                    if j == 0:
                        yt = yp.tile([P, G * C], BF16, tag="y", bufs=3)
                    nc.vector.tensor_copy(out=yt[:, j * C:(j + 1) * C], in_=psy[:])
                    if j == g_here - 1:
                        g0 = mh - (g_here - 1)
                        # Mid-kernel stores ride the idle gpsimd SWDGE ring
                        # (never queue behind loads). The last slot's stores
                        # use the scalar HWDGE ring instead: loads are done
                        # by then, and keeping SWDGE idle at the end makes
                        # the epilogue's gpsimd drain instant.
                        eng = nc.gpsimd if s + 1 < SLOTS else nc.scalar
                        eng.dma_start(
                            y_d[s, :, g0 * C:(g0 + g_here) * C],
                            yt[:, : g_here * C],
                        )

    nc.compile()
    return nc


def _route(x, gate_w):
    """Replicate the reference router. Returns (topk_idx, topk_w) as numpy."""
    try:
        import jax
        import jax.numpy as jnp

        cpu = jax.devices("cpu")[0]
        with jax.default_device(cpu):
            logits = jnp.asarray(x) @ jnp.asarray(gate_w)
            probs = jax.nn.softmax(logits.astype(jnp.float32), axis=-1)
            topk_w, topk_idx = jax.lax.top_k(probs, K)
            topk_w = topk_w / jnp.sum(topk_w, axis=-1, keepdims=True)
            return np.asarray(topk_idx), np.asarray(topk_w)
    except Exception:
        logits = x.astype(np.float32) @ gate_w.astype(np.float32)
        lm = logits.max(-1, keepdims=True)
        p = np.exp(logits - lm)
        p /= p.sum(-1, keepdims=True)
        topk_idx = np.argsort(-p, kind="stable", axis=-1)[:, :K]
        topk_w = np.take_along_axis(p, topk_idx, axis=-1)
        topk_w = topk_w / topk_w.sum(-1, keepdims=True)
        return topk_idx.astype(np.int32), topk_w


def _silu(v):
    return v / (1.0 + np.exp(-v))


def _pack_pm(a, kt):
    """[S, kt*P, free] -> partition-major [S, P, kt*free]."""
    s, rows, free = a.shape
    return np.ascontiguousarray(
        a.reshape(s, kt, P, free).transpose(0, 2, 1, 3).reshape(s, P, kt * free)
    )


def kernel(hidden_states, gate_w, w_gate_proj, w_up_proj, w_down_proj):
    global LAST_RESULT, _NC_CACHE

    x = np.asarray(hidden_states, dtype=np.float32)
    gate_w = np.asarray(gate_w, dtype=np.float32)
    wg_all = np.asarray(w_gate_proj, dtype=np.float32)
    wu_all = np.asarray(w_up_proj, dtype=np.float32)
    wd_all = np.asarray(w_down_proj, dtype=np.float32)

    # ---- Host router ----
    topk_idx, topk_w = _route(x, gate_w)

    # Per-expert token lists (kept on device up to capacity C; rest on host)
    route_w = np.zeros((T, E), np.float32)
    np.put_along_axis(route_w, topk_idx, topk_w.astype(np.float32), axis=-1)
    expert_tokens = [np.nonzero(route_w[:, e])[0] for e in range(E)]

    x_bf = x.astype(ml_dtypes.bfloat16)

    # ---- Build per-core inputs (partition-major packing) ----
    in_maps = []
    for core in range(NCORES):
        experts = [core + NCORES * s for s in range(SLOTS)]
        xt = np.zeros((SLOTS, H, C), ml_dtypes.bfloat16)
        for s, e in enumerate(experts):
            idx = expert_tokens[e][:C]
            xt[s, :, : len(idx)] = x_bf[idx].T
        in_maps.append(
            {
                "xt": _pack_pm(xt, KH),
                "wg": _pack_pm(wg_all[experts].astype(ml_dtypes.bfloat16), KH),
                "wu": _pack_pm(wu_all[experts].astype(ml_dtypes.bfloat16), KH),
                "wd": _pack_pm(wd_all[experts].astype(ml_dtypes.bfloat16), KF),
            }
        )

    # ---- Device run ----
    if _NC_CACHE is None:
        _NC_CACHE = _build_graph()
    nc = _NC_CACHE
    res = run_bass_kernel_spmd(nc, in_maps, core_ids=list(range(NCORES)))
    LAST_RESULT = res

    # ---- Host combine ----
    out = np.zeros((T, H), np.float32)
    for e in range(E):
        core, s = e % NCORES, e // NCORES
        idx = expert_tokens[e]
        kept, ov = idx[:C], idx[C:]
        ysl = np.asarray(res.results[core]["y"][s])  # [P, MH*C] bf16
        yT = (
            ysl.reshape(P, MH, C).transpose(1, 0, 2).reshape(H, C)
            .astype(np.float32)
        )
        w_kept = route_w[kept, e]
        out[kept] += w_kept[:, None] * yT[:, : len(kept)].T
        if len(ov):
            xo = x[ov]
            h = _silu(xo @ wg_all[e]) * (xo @ wu_all[e])
            out[ov] += route_w[ov, e][:, None] * (h @ wd_all[e])

    return out
